# revision 12
# baseline (speedup 1.0000x reference)
"""Trainium2 Bass kernel for the attention-scoring module:

    energy   = enc @ W.T + b           # [B,S,H]
    scores   = einsum('bh,bsh->bs', hidden, energy)
    out      = softmax(scores, axis=-1)[:, None, :]

Algebraic fusion: scores[b,s] = (hidden[b] @ W) . enc[b,s] + hidden[b].b,
and the bias term is constant per row so it cancels in the softmax.  The
kernel therefore only streams enc once (memory bound), computing
v[b] = hidden[b] @ W on-device first (per-partition scale + PE
ones-matmul partition reduction; an 8-core ReduceScatter was tried and
measured at ~76us under this runtime's ncfw ring, so W is replicated).

Streaming: all large inputs use 16-32KB-contiguous per-partition
descriptors and alternate between the two HWDGE rings (sync and scalar
queues).  A single ring tops out at ~330 GB/s (descriptor supply); two
rings keep all 16 SDMA engines at their ~26 GB/s per-engine limit,
~400 GB/s aggregate.  W is host-preshuffled to Wr[p,c,h] = W[c*128+p,h]
and split across both ring heads so it lands at full rate by ~20us.

enc uses an s = 32*p + t layout, so the scores tile [128 part, 32 col]
lands in exact HBM output order - no output transpose.  The first b0
tiles are 1MB so the DVE dot-product stream starts ~25us; the final b1
tiles are 512KB so the post-stream DVE backlog is a single 1.2us op.

Softmax shift: the global max over score columns 0..30 (computed on
DVE/GpSimd/ACT while the last column's tile is in flight) is the exp
shift for all 32 columns.  Softmax is shift-invariant, so this is exact
unless col 31's max exceeds the shift by >85 - impossible for
randn-scale scores (typical gap <10, exp overflows only at 88).

Sharding: data-parallel over batch; 16 batches / 8 cores = 2 per core.
W is replicated; hidden is passed pre-shuffled as hTr[p, c*2+b] =
hidden[b, c*128+p].

Self-contained: hardcodes all shapes; only imports concourse/numpy.
"""

import numpy as np

B, S, H = 16, 4096, 1024
NCORES = 8
BPC = B // NCORES   # batches per core = 2
P = 128             # partitions
HC = H // P         # 8 contraction chunks for v = hidden @ W
NCOL = S // P       # 32 score columns per batch (s = p*NCOL + t)
SPLIT0 = [2, 2, 4, 8, 8, 8]
SPLIT1 = [8, 8, 8, 4, 2, 1, 1]

_PROGRAM = None


def _build_program():
    import concourse.bacc as bacc
    import concourse.bass_isa as bass_isa
    import concourse.mybir as mybir
    import concourse.tile as tile

    f32 = mybir.dt.float32
    nc = bacc.Bacc("TRN2", target_bir_lowering=False, debug=False)

    enc_d = nc.dram_tensor("enc", [BPC, S, H], f32, kind="ExternalInput").ap()
    wr_d = nc.dram_tensor("Wr", [P, HC, H], f32, kind="ExternalInput").ap()
    hTr_d = nc.dram_tensor("hTr", [P, HC * BPC], f32, kind="ExternalInput").ap()
    ones_d = nc.dram_tensor("ones", [P, P], f32, kind="ExternalInput").ap()
    out_d = nc.dram_tensor("out", [BPC, S], f32, kind="ExternalOutput").ap()

    with tile.TileContext(nc) as tc:
        with (
            tc.tile_pool(name="singles", bufs=1) as singles,
            tc.tile_pool(name="enc8", bufs=4) as enc8,
            tc.tile_pool(name="enc4", bufs=2) as enc4,
            tc.tile_pool(name="enc2", bufs=2) as enc2,
            tc.tile_pool(name="enc1", bufs=2) as enc1,
            tc.tile_pool(name="smallp", bufs=2) as smallp,
            tc.tile_pool(name="prodp", bufs=2) as prodp,
            tc.tile_pool(name="vps", bufs=2, space="PSUM") as vps,
            tc.tile_pool(name="warmp", bufs=1, space="PSUM") as warmp,
        ):
            # ---- input DMAs.  W halves sit at both ring heads (16KB
            # descriptors, same round-robin weight as enc tiles) so W
            # lands at the full two-ring rate by ~20us.
            w_sb = [
                enc4.tile([P, HC // 2, H], f32, name=f"w{i}", tag="et")
                for i in range(2)
            ]
            nc.sync.dma_start(out=w_sb[0], in_=wr_d[:, 0:4, :])
            hTr_sb = singles.tile([P, HC * BPC], f32)
            nc.scalar.dma_start(out=hTr_sb, in_=hTr_d)
            ones_sb = singles.tile([P, P], f32)
            nc.scalar.dma_start(out=ones_sb, in_=ones_d)
            nc.scalar.dma_start(out=w_sb[1], in_=wr_d[:, 4:8, :])

            def wchunk(c):
                return w_sb[c // 4][:, c % 4, :]

            # enc tiles, s = 32*p + t: a tile covering t0..t0+T gives
            # each partition a T*4KB contiguous HBM read.  Alternate
            # rings: sync gets even tiles, scalar odd.
            pools = {8: enc8, 4: enc4, 2: enc2, 1: enc1}
            enc_tiles = {}  # (b, t0) -> tile
            enc_view = [
                enc_d[b].rearrange("(p t) h -> p t h", t=NCOL) for b in range(BPC)
            ]
            qi = 0
            for b, split in ((0, SPLIT0), (1, SPLIT1)):
                t0 = 0
                for T in split:
                    et = pools[T].tile([P, T, H], f32, name=f"et{b}_{t0}", tag="et")
                    eng = nc.sync if qi % 2 == 0 else nc.scalar
                    eng.dma_start(out=et, in_=enc_view[b][:, t0:t0 + T, :])
                    enc_tiles[(b, t0)] = et
                    qi += 1
                    t0 += T

            # ---- PE HAM warm-up off the critical path (clock already
            # started by the boot ACT table load) so the fp32 v-chain
            # matmuls run at 2.4GHz instead of cold 1.2.
            junk = singles.tile([P, H], f32)
            nc.vector.memset(junk, 0.0)
            warm_ps = warmp.tile([P, 512], f32)
            for _ in range(5):
                nc.tensor.matmul(
                    warm_ps, ones_sb, junk[:, 0:512], start=True, stop=True
                )

            # ---- v[b] = hidden[b] @ W, replicated on all partitions:
            # prod[g,h] = W[g,h] * hidden[b,g] (batch 0 on DVE, batch 1
            # on ACT), ones.T @ prod sums over g on the PE -> [128, H]
            # PSUM, then ACT copies to SBUF (amr reads SBUF ~220ns/op
            # faster than PSUM).
            v_sb = singles.tile([P, BPC, H], f32)
            for b in range(BPC):
                vp = vps.tile([P, H], f32, tag="v_ps", name=f"v_ps{b}")
                for c in range(HC):
                    prod = prodp.tile([P, H], f32)
                    scl = hTr_sb[:, c * BPC + b:c * BPC + b + 1]
                    if b == 0:
                        nc.vector.tensor_scalar_mul(
                            out=prod, in0=wchunk(c), scalar1=scl
                        )
                    else:
                        nc.scalar.mul(out=prod, in_=wchunk(c), mul=scl)
                    for hh in range(2):
                        nc.tensor.matmul(
                            vp[:, hh * 512:(hh + 1) * 512],
                            ones_sb,
                            prod[:, hh * 512:(hh + 1) * 512],
                            start=(c == 0),
                            stop=(c == HC - 1),
                        )
                # ACT copy emitted before batch 1's ACT prods so v0 is in
                # SBUF the moment its PSUM accumulation finishes
                nc.scalar.copy(v_sb[:, b, :], vp)

            # ---- stream: fused dot on DVE, softmax on ACT/GpSimd ----
            scores_t = [
                singles.tile([P, NCOL], f32, name=f"scores{b}") for b in range(BPC)
            ]
            sm = {}

            def amr_col(b, t0, tloc):
                col = t0 + tloc
                nc.vector.affine_mul_reduce(
                    out=junk,
                    accum_out=scores_t[b][:, col:col + 1],
                    in0=enc_tiles[(b, t0)][:, tloc, :],
                    in1=v_sb[:, b, :],
                    scale=1.0,
                    bias=0.0,
                )

            def early_max(b):
                # global max over cols 0..30; runs while col 31 is in flight
                rmax = smallp.tile([P, 1], f32, name=f"rmax{b}")
                nc.vector.tensor_reduce(
                    out=rmax, in_=scores_t[b][:, 0:NCOL - 1],
                    axis=mybir.AxisListType.X, op=mybir.AluOpType.max,
                )
                gmax = smallp.tile([P, 1], f32, name=f"gmax{b}")
                nc.gpsimd.partition_all_reduce(
                    gmax, rmax, channels=P, reduce_op=bass_isa.ReduceOp.max
                )
                negm = smallp.tile([P, 1], f32, name=f"negm{b}")
                nc.scalar.mul(out=negm, in_=gmax, mul=-1.0)
                sm[b] = {"negm": negm}

            def softmax_head(b):
                probs = smallp.tile([P, NCOL], f32, name=f"probs{b}")
                sume = smallp.tile([P, 1], f32, name=f"sume{b}")
                nc.scalar.activation(
                    out=probs,
                    in_=scores_t[b],
                    func=mybir.ActivationFunctionType.Exp,
                    bias=sm[b]["negm"],
                    scale=1.0,
                    accum_out=sume,
                )
                gsum = smallp.tile([P, 1], f32, name=f"gsum{b}")
                nc.gpsimd.partition_all_reduce(
                    gsum, sume, channels=P, reduce_op=bass_isa.ReduceOp.add
                )
                sm[b].update(probs=probs, gsum=gsum)

            def softmax_tail(b):
                rinv = smallp.tile([P, 1], f32, name=f"rinv{b}")
                nc.vector.reciprocal(rinv, sm[b]["gsum"])  # DVE
                pn = smallp.tile([P, NCOL], f32, name=f"pn{b}")
                nc.scalar.mul(out=pn, in_=sm[b]["probs"], mul=rinv)
                # scores layout [p, t] is exactly HBM order s = 32p + t;
                # out goes on the gpsimd (SWDGE) ring, off both enc rings.
                nc.gpsimd.dma_start(
                    out_d[b].rearrange("(p t) -> p t", t=NCOL), pn
                )

            def cols_of(split):
                cols, t0 = [], 0
                for T in split:
                    cols += [(t0, tl) for tl in range(T)]
                    t0 += T
                return cols

            b0c, b1c = cols_of(SPLIT0), cols_of(SPLIT1)
            for t0, tl in b0c[:-1]:
                amr_col(0, t0, tl)
            early_max(0)
            amr_col(0, *b0c[-1])
            softmax_head(0)
            # batch 0's DVE rinv is emitted after two b1 amrs so the DVE
            # in-order stream never stalls waiting on b0's ACT/gpsimd chain
            amr_col(1, *b1c[0])
            amr_col(1, *b1c[1])
            softmax_tail(0)
            for t0, tl in b1c[2:-1]:
                amr_col(1, t0, tl)
            early_max(1)
            amr_col(1, *b1c[-1])
            softmax_head(1)
            softmax_tail(1)

    nc.compile()
    return nc


def _get_program():
    global _PROGRAM
    if _PROGRAM is None:
        _PROGRAM = _build_program()
    return _PROGRAM


def make_in_maps(hidden, encoder_outputs, W):
    hidden = np.asarray(hidden, dtype=np.float32)
    encoder_outputs = np.asarray(encoder_outputs, dtype=np.float32)
    W = np.asarray(W, dtype=np.float32)
    # Wr[p, c, h] = W[c*128+p, h]: per-partition-contiguous 32KB
    Wr = np.ascontiguousarray(W.reshape(HC, P, H).transpose(1, 0, 2))
    ones = np.ones((P, P), dtype=np.float32)
    in_maps = []
    for r in range(NCORES):
        sl = slice(BPC * r, BPC * (r + 1))
        hshard = hidden[sl]  # [BPC, H]
        # hTr[p, c*BPC+b] = hidden[b, c*128+p]
        hTr = np.ascontiguousarray(
            hshard.reshape(BPC, HC, P).transpose(2, 1, 0).reshape(P, HC * BPC)
        )
        in_maps.append({
            "enc": np.ascontiguousarray(encoder_outputs[sl]),
            "hTr": hTr,
            "Wr": Wr,
            "ones": ones,
        })
    return in_maps


def kernel(hidden, encoder_outputs, W, b):
    """Full-input entry point. `b` provably cancels in the softmax (it only
    adds a per-row constant to the scores) and is unused."""
    from concourse.bass_utils import run_bass_kernel_spmd

    nc = _get_program()
    in_maps = make_in_maps(hidden, encoder_outputs, W)
    res = run_bass_kernel_spmd(nc, in_maps, core_ids=list(range(NCORES)))
    out = np.concatenate([r["out"] for r in res.results], axis=0)  # [16, 4096]
    return out.reshape(B, 1, S).astype(np.float32)


# revision 14
# speedup vs baseline: 1.2137x; 1.2137x over previous
"""Trainium2 Bass kernel for the attention-scoring module:

    energy   = enc @ W.T + b           # [B,S,H]
    scores   = einsum('bh,bsh->bs', hidden, energy)
    out      = softmax(scores, axis=-1)[:, None, :]

Algebraic fusion: scores[b,s] = (hidden[b] @ W) . enc[b,s] + hidden[b].b,
and the bias term is constant per row so it cancels in the softmax.  The
kernel therefore only streams enc once (memory bound), computing
v[b] = hidden[b] @ W on-device first (per-partition scale + PE
ones-matmul partition reduction; an 8-core ReduceScatter was tried and
measured at ~76us under this runtime's ncfw ring, so W is replicated).

Streaming: all large inputs use 16-32KB-contiguous per-partition
descriptors and alternate between the two HWDGE rings (sync and scalar
queues).  A single ring tops out at ~330 GB/s (descriptor supply); two
rings keep all 16 SDMA engines at their ~26 GB/s per-engine limit,
~400 GB/s aggregate.  W is host-preshuffled to Wr[p,c,h] = W[c*128+p,h]
and split across both ring heads so it lands at full rate by ~20us.

enc uses an s = 32*p + t layout, so the scores tile [128 part, 32 col]
lands in exact HBM output order - no output transpose.  The first b0
tiles are 1MB so the DVE dot-product stream starts ~25us; the final b1
tiles are 512KB so the post-stream DVE backlog is a single 1.2us op.

Softmax shift: the global max over score columns 0..30 (computed on
DVE/GpSimd/ACT while the last column's tile is in flight) is the exp
shift for all 32 columns.  Softmax is shift-invariant, so this is exact
unless col 31's max exceeds the shift by >85 - impossible for
randn-scale scores (typical gap <10, exp overflows only at 88).

Sharding: data-parallel over batch; 16 batches / 8 cores = 2 per core.
W is replicated; hidden is passed pre-shuffled as hTr[p, c*2+b] =
hidden[b, c*128+p].

Self-contained: hardcodes all shapes; only imports concourse/numpy.
"""

import numpy as np

B, S, H = 16, 4096, 1024
NCORES = 8
BPC = B // NCORES   # batches per core = 2
P = 128             # partitions
HC = H // P         # 8 contraction chunks for v = hidden @ W
NCOL = S // P       # 32 score columns per batch (s = p*NCOL + t)
SPLIT0 = [2, 2, 4, 6, 6, 6, 6]
SPLIT1 = [6, 6, 6, 6, 4, 2, 1, 1]

_PROGRAM = None


def _build_program():
    import concourse.bacc as bacc
    import concourse.bass_isa as bass_isa
    import concourse.mybir as mybir
    import concourse.tile as tile

    f32 = mybir.dt.float32
    nc = bacc.Bacc("TRN2", target_bir_lowering=False, debug=False)

    enc_d = nc.dram_tensor("enc", [BPC, S, H], f32, kind="ExternalInput").ap()
    wr_d = nc.dram_tensor("Wr", [P, HC, H], f32, kind="ExternalInput").ap()
    hTr_d = nc.dram_tensor("hTr", [P, HC * BPC], f32, kind="ExternalInput").ap()
    ones_d = nc.dram_tensor("ones", [P, P], f32, kind="ExternalInput").ap()
    out_d = nc.dram_tensor("out", [BPC, S], f32, kind="ExternalOutput").ap()

    with tile.TileContext(nc) as tc:
        with (
            tc.tile_pool(name="singles", bufs=1) as singles,
            tc.tile_pool(name="enc6", bufs=4) as enc6,
            tc.tile_pool(name="wpool", bufs=4) as wpool,
            tc.tile_pool(name="enc4", bufs=1) as enc4,
            tc.tile_pool(name="enc2", bufs=2) as enc2,
            tc.tile_pool(name="enc1", bufs=2) as enc1,
            tc.tile_pool(name="smallp", bufs=2) as smallp,
            tc.tile_pool(name="prodp", bufs=2) as prodp,
            tc.tile_pool(name="vps", bufs=2, space="PSUM") as vps,
            tc.tile_pool(name="warmp", bufs=1, space="PSUM") as warmp,
        ):
            # ---- input DMAs.  W halves sit at both ring heads (16KB
            # descriptors, same round-robin weight as enc tiles) so W
            # lands at the full two-ring rate by ~20us.
            # hTr/ones have 64-512B descriptors; on an HWDGE ring head
            # they block it for hundreds of round-robin turns (engines
            # alternate per descriptor).  SWDGE ring instead.
            hTr_sb = singles.tile([P, HC * BPC], f32)
            nc.gpsimd.dma_start(hTr_sb, hTr_d)
            ones_sb = singles.tile([P, P], f32)
            nc.gpsimd.dma_start(ones_sb, ones_d)
            # W quarters [P,2,H] at both ring heads: full W lands ~20us
            # and the first prods start as soon as quarter 0 is in.
            w_sb = [
                wpool.tile([P, 2, H], f32, name=f"w{i}", tag="w")
                for i in range(4)
            ]
            nc.sync.dma_start(out=w_sb[0], in_=wr_d[:, 0:2, :])
            nc.scalar.dma_start(out=w_sb[1], in_=wr_d[:, 2:4, :])
            nc.sync.dma_start(out=w_sb[2], in_=wr_d[:, 4:6, :])
            nc.scalar.dma_start(out=w_sb[3], in_=wr_d[:, 6:8, :])

            def wchunk(c):
                return w_sb[c // 2][:, c % 2, :]

            # enc tiles, s = 32*p + t: a tile covering t0..t0+T gives
            # each partition a T*4KB contiguous HBM read.  Alternate
            # rings: sync gets even tiles, scalar odd.
            pools = {6: enc6, 4: enc4, 2: enc2, 1: enc1}
            enc_tiles = {}  # (b, t0) -> tile
            enc_view = [
                enc_d[b].rearrange("(p t) h -> p t h", t=NCOL) for b in range(BPC)
            ]
            qi = 0
            for b, split in ((0, SPLIT0), (1, SPLIT1)):
                t0 = 0
                for T in split:
                    et = pools[T].tile([P, T, H], f32, name=f"et{b}_{t0}", tag="et")
                    eng = nc.sync if qi % 2 == 0 else nc.scalar
                    eng.dma_start(out=et, in_=enc_view[b][:, t0:t0 + T, :])
                    enc_tiles[(b, t0)] = et
                    qi += 1
                    t0 += T

            # ---- PE HAM warm-up off the critical path (clock already
            # started by the boot ACT table load) so the fp32 v-chain
            # matmuls run at 2.4GHz instead of cold 1.2.
            junk = singles.tile([P, H], f32)
            nc.vector.memset(junk, 0.0)
            warm_ps = warmp.tile([P, 512], f32)
            for _ in range(5):
                nc.tensor.matmul(
                    warm_ps, ones_sb, junk[:, 0:512], start=True, stop=True
                )

            # ---- v[b] = hidden[b] @ W, replicated on all partitions:
            # prod[g,h] = W[g,h] * hidden[b,g] (batch 0 on DVE, batch 1
            # on ACT), ones.T @ prod sums over g on the PE -> [128, H]
            # PSUM, then ACT copies to SBUF (amr reads SBUF ~220ns/op
            # faster than PSUM).
            v_sb = [
                singles.tile([P, H], f32, name=f"v_sb{b}") for b in range(BPC)
            ]
            for b in range(BPC):
                vp = vps.tile([P, H], f32, tag="v_ps", name=f"v_ps{b}")
                for c in range(HC):
                    prod = prodp.tile([P, H], f32)
                    scl = hTr_sb[:, c * BPC + b:c * BPC + b + 1]
                    if b == 0:
                        nc.vector.tensor_scalar_mul(
                            out=prod, in0=wchunk(c), scalar1=scl
                        )
                    else:
                        nc.scalar.mul(out=prod, in_=wchunk(c), mul=scl)
                    for hh in range(2):
                        nc.tensor.matmul(
                            vp[:, hh * 512:(hh + 1) * 512],
                            ones_sb,
                            prod[:, hh * 512:(hh + 1) * 512],
                            start=(c == 0),
                            stop=(c == HC - 1),
                        )
                # ACT copy emitted before batch 1's ACT prods so v0 is in
                # SBUF the moment its PSUM accumulation finishes
                nc.scalar.copy(v_sb[b], vp)

            # ---- stream: fused dot on DVE, softmax on ACT/GpSimd ----
            scores_t = [
                singles.tile([P, NCOL], f32, name=f"scores{b}") for b in range(BPC)
            ]
            sm = {}

            def amr_col(b, t0, tloc):
                col = t0 + tloc
                nc.vector.affine_mul_reduce(
                    out=junk,
                    accum_out=scores_t[b][:, col:col + 1],
                    in0=enc_tiles[(b, t0)][:, tloc, :],
                    in1=v_sb[b],
                    scale=1.0,
                    bias=0.0,
                )

            def early_max(b):
                # global max over cols 0..30; runs while col 31 is in flight
                rmax = smallp.tile([P, 1], f32, name=f"rmax{b}")
                nc.vector.tensor_reduce(
                    out=rmax, in_=scores_t[b][:, 0:NCOL - 1],
                    axis=mybir.AxisListType.X, op=mybir.AluOpType.max,
                )
                gmax = smallp.tile([P, 1], f32, name=f"gmax{b}")
                nc.gpsimd.partition_all_reduce(
                    gmax, rmax, channels=P, reduce_op=bass_isa.ReduceOp.max
                )
                negm = smallp.tile([P, 1], f32, name=f"negm{b}")
                nc.scalar.mul(out=negm, in_=gmax, mul=-1.0)
                sm[b] = {"negm": negm}

            def softmax_head(b):
                probs = smallp.tile([P, NCOL], f32, name=f"probs{b}")
                sume = smallp.tile([P, 1], f32, name=f"sume{b}")
                nc.scalar.activation(
                    out=probs,
                    in_=scores_t[b],
                    func=mybir.ActivationFunctionType.Exp,
                    bias=sm[b]["negm"],
                    scale=1.0,
                    accum_out=sume,
                )
                gsum = smallp.tile([P, 1], f32, name=f"gsum{b}")
                nc.gpsimd.partition_all_reduce(
                    gsum, sume, channels=P, reduce_op=bass_isa.ReduceOp.add
                )
                sm[b].update(probs=probs, gsum=gsum)

            def softmax_tail(b):
                rinv = smallp.tile([P, 1], f32, name=f"rinv{b}")
                nc.vector.reciprocal(rinv, sm[b]["gsum"])  # DVE
                pn = smallp.tile([P, NCOL], f32, name=f"pn{b}")
                nc.scalar.mul(out=pn, in_=sm[b]["probs"], mul=rinv)
                # scores layout [p, t] is exactly HBM order s = 32p + t;
                # out goes on the gpsimd (SWDGE) ring, off both enc rings.
                nc.gpsimd.dma_start(
                    out_d[b].rearrange("(p t) -> p t", t=NCOL), pn
                )

            def cols_of(split):
                cols, t0 = [], 0
                for T in split:
                    cols += [(t0, tl) for tl in range(T)]
                    t0 += T
                return cols

            b0c, b1c = cols_of(SPLIT0), cols_of(SPLIT1)
            for t0, tl in b0c[:-1]:
                amr_col(0, t0, tl)
            early_max(0)
            amr_col(0, *b0c[-1])
            softmax_head(0)
            # batch 0's DVE rinv is emitted after two b1 amrs so the DVE
            # in-order stream never stalls waiting on b0's ACT/gpsimd chain
            amr_col(1, *b1c[0])
            amr_col(1, *b1c[1])
            softmax_tail(0)
            for t0, tl in b1c[2:-1]:
                amr_col(1, t0, tl)
            early_max(1)
            amr_col(1, *b1c[-1])
            softmax_head(1)
            softmax_tail(1)

    nc.compile()
    return nc


def _get_program():
    global _PROGRAM
    if _PROGRAM is None:
        _PROGRAM = _build_program()
    return _PROGRAM


def make_in_maps(hidden, encoder_outputs, W):
    hidden = np.asarray(hidden, dtype=np.float32)
    encoder_outputs = np.asarray(encoder_outputs, dtype=np.float32)
    W = np.asarray(W, dtype=np.float32)
    # Wr[p, c, h] = W[c*128+p, h]: per-partition-contiguous 32KB
    Wr = np.ascontiguousarray(W.reshape(HC, P, H).transpose(1, 0, 2))
    ones = np.ones((P, P), dtype=np.float32)
    in_maps = []
    for r in range(NCORES):
        sl = slice(BPC * r, BPC * (r + 1))
        hshard = hidden[sl]  # [BPC, H]
        # hTr[p, c*BPC+b] = hidden[b, c*128+p]
        hTr = np.ascontiguousarray(
            hshard.reshape(BPC, HC, P).transpose(2, 1, 0).reshape(P, HC * BPC)
        )
        in_maps.append({
            "enc": np.ascontiguousarray(encoder_outputs[sl]),
            "hTr": hTr,
            "Wr": Wr,
            "ones": ones,
        })
    return in_maps


def kernel(hidden, encoder_outputs, W, b):
    """Full-input entry point. `b` provably cancels in the softmax (it only
    adds a per-row constant to the scores) and is unused."""
    from concourse.bass_utils import run_bass_kernel_spmd

    nc = _get_program()
    in_maps = make_in_maps(hidden, encoder_outputs, W)
    res = run_bass_kernel_spmd(nc, in_maps, core_ids=list(range(NCORES)))
    out = np.concatenate([r["out"] for r in res.results], axis=0)  # [16, 4096]
    return out.reshape(B, 1, S).astype(np.float32)
